# revision 9
# baseline (speedup 1.0000x reference)
"""Trainium2 Bass kernel for nn_ODEG_8942121911067 (gnn_message_passing).

Math (derived from the reference ODE block; the Euler loop collapses to
its last step since f is recomputed from x_aug every iteration):

    out = relu(0.5*x_aug + 0.125*sigmoid(alpha)_i * (adj @ x_aug)
               + 0.25*S*R + 0.25*(x_aug @_t W2mix))

with x_aug = concat([x, zeros10], -1), S[b,n,t] = sum_f x_aug[b,n,t,f],
R[m] = sum_n ((w*clip(d,0,1)) @ w.T)[m,n], W2mix = (w2*clip(d2,0,1)) @ w2.T.

Device strategy (data-parallel over batch, 4 batches/core on 8 cores).
Two compiled variants, dispatched on the runtime structure of W2mix:

General path (dense W2mix): the node term runs as one K=512 matmul per
output tile (stationary A = 0.125*diag(sigmoid(alpha)) @ adj, bf16 -
the adjacency term is ~1% of output magnitude so bf16 there is ~1e-6
overall). All precision-critical terms (0.5*x + temporal mix + rank-1
body, which are layout-hostile to the PE) fold host-side into a fp32
side tensor q whose 65th column carries S; the DVE adds q during PSUM
eviction, pad cols are a stride-0-broadcast outer product, ACT applies
relu. ~34 MB HBM/core.

Fast path (W2mix == c*I, e.g. the eye-initialized reference weights):
the temporal mix degenerates to c*x, so no q is needed: x travels once
as fp32, feeding the adjacency matmul (fp32r, full PE rate), the exact
alpha*x = (0.5+0.25c)*x eviction add on the DVE, and the rank-1 term
rides the same PSUM group as a K=24 matmul against S^T. ~29 MB HBM/core.
"""

import numpy as np

B, N, T, F = 32, 512, 24, 64
NUM_ZEROS = 10
FA = F + NUM_ZEROS  # 74
FQ = F + 1  # q carries 64 real cols + one S column (general path)
N_CORES = 8
BPC = B // N_CORES  # batches per core = 4
NT = N // 128  # node chunks = 4
NCH = (T * F) // 512  # moving-dim chunks of 512 = 3
TPC = 512 // F  # t-values per 512-chunk = 8

_CACHE = {}


def _mk_nc():
    from concourse import bacc

    return bacc.Bacc("TRN2", target_bir_lowering=False, debug=False,
                     num_devices=N_CORES)


def _build_general():
    import concourse.mybir as mybir
    import concourse.tile as tile

    bf16 = mybir.dt.bfloat16
    f32 = mybir.dt.float32

    nc = _mk_nc()
    x_d = nc.dram_tensor("xin", [BPC, N, T, F], bf16, kind="ExternalInput").ap()
    q_d = nc.dram_tensor("q", [BPC, N, T, FQ], f32, kind="ExternalInput").ap()
    at_d = nc.dram_tensor("at", [N, N], bf16, kind="ExternalInput").ap()
    rp_d = nc.dram_tensor("rp", [128, NUM_ZEROS], f32, kind="ExternalInput").ap()
    out_d = nc.dram_tensor("out", [BPC, N, T, FA], f32, kind="ExternalOutput").ap()

    with tile.TileContext(nc) as tc:
        with (
            tc.tile_pool(name="const", bufs=1) as cpool,
            tc.tile_pool(name="xp", bufs=2 * NT) as xpool,
            tc.tile_pool(name="qp", bufs=4) as qpool,
            tc.tile_pool(name="op", bufs=4) as opool,
            tc.tile_pool(name="ps", bufs=8, space="PSUM") as pspool,
        ):
            rp = cpool.tile([128, 1, NUM_ZEROS], f32, tag="rp")
            nc.scalar.dma_start(rp[:], rp_d[:].rearrange("p (a b) -> p a b", a=1))
            at_sb = []
            for kc in range(NT):
                a = cpool.tile([128, N], bf16, tag=f"at{kc}")
                nc.scalar.dma_start(a[:], at_d[kc * 128:(kc + 1) * 128, :])
                at_sb.append(a)

            for b in range(BPC):
                xts = []
                for kc in range(NT):
                    xt = xpool.tile([128, T, F], bf16, tag="xt")
                    nc.sync.dma_start(xt[:], x_d[b, kc * 128:(kc + 1) * 128])
                    xts.append(xt.rearrange("p a b -> p (a b)"))
                for ic in range(NT):
                    qt = qpool.tile([128, T, FQ], f32, tag="qt")
                    nc.sync.dma_start(qt[:], q_d[b, ic * 128:(ic + 1) * 128])
                    ot = opool.tile([128, T, FA], f32, tag="ot")
                    for nch in range(NCH):
                        ps = pspool.tile([128, 512], f32, tag="ps")
                        for kc in range(NT):
                            nc.tensor.matmul(
                                ps[:],
                                at_sb[kc][:, ic * 128:(ic + 1) * 128],
                                xts[kc][:, nch * 512:(nch + 1) * 512],
                                start=(kc == 0),
                                stop=(kc == NT - 1),
                            )
                        t0 = nch * TPC
                        nc.vector.scalar_tensor_tensor(
                            ot[:, t0:t0 + TPC, 0:F],
                            ps[:].rearrange("p (a b) -> p a b", a=TPC),
                            1.0,
                            qt[:, t0:t0 + TPC, 0:F],
                            mybir.AluOpType.mult,
                            mybir.AluOpType.add,
                        )
                    # pad cols: outer product S[p,t] * 0.25*R[f] in one DVE
                    # op via stride-0 broadcast APs; relu folds into ACT below
                    nc.vector.scalar_tensor_tensor(
                        ot[:, :, F:FA],
                        qt[:, :, F:FQ].broadcast_to([128, T, NUM_ZEROS]),
                        1.0,
                        rp[:].broadcast_to([128, T, NUM_ZEROS]),
                        mybir.AluOpType.mult,
                        mybir.AluOpType.mult,
                    )
                    nc.scalar.activation(ot[:], ot[:],
                                         mybir.ActivationFunctionType.Relu)
                    nc.scalar.dma_start(out_d[b, ic * 128:(ic + 1) * 128], ot[:])

    nc.compile()
    return nc


def _build_fast():
    import concourse.mybir as mybir
    import concourse.tile as tile

    f32 = mybir.dt.float32
    f32r = mybir.dt.float32r

    nc = _mk_nc()
    x_d = nc.dram_tensor("xin", [BPC, N, T, F], f32r, kind="ExternalInput").ap()
    at_d = nc.dram_tensor("at", [N, N], f32r, kind="ExternalInput").ap()
    # S^T per batch (rank-1 stationary), S per batch (pad broadcast),
    # Vbody[t', t*F+f] = delta[t',t]*0.25*R[f], alpha_rep = 0.5+0.25c,
    # rp = 0.25*R[64:74] replicated
    stv_d = nc.dram_tensor("stv", [BPC, T, N], f32r, kind="ExternalInput").ap()
    sp_d = nc.dram_tensor("sp", [BPC, N, T], f32, kind="ExternalInput").ap()
    vb_d = nc.dram_tensor("vb", [T, T * F], f32r, kind="ExternalInput").ap()
    al_d = nc.dram_tensor("al", [128, 1], f32, kind="ExternalInput").ap()
    rp_d = nc.dram_tensor("rp", [128, NUM_ZEROS], f32, kind="ExternalInput").ap()
    out_d = nc.dram_tensor("out", [BPC, N, T, FA], f32, kind="ExternalOutput").ap()

    with tile.TileContext(nc) as tc:
        with (
            tc.tile_pool(name="const", bufs=1) as cpool,
            tc.tile_pool(name="xp", bufs=2 * NT) as xpool,
            tc.tile_pool(name="sp", bufs=2) as spool,
            tc.tile_pool(name="pp", bufs=4) as ppool,
            tc.tile_pool(name="op", bufs=4) as opool,
            tc.tile_pool(name="ps", bufs=8, space="PSUM") as pspool,
        ):
            rp = cpool.tile([128, 1, NUM_ZEROS], f32, tag="rp")
            nc.scalar.dma_start(rp[:], rp_d[:].rearrange("p (a b) -> p a b", a=1))
            al = cpool.tile([128, 1], f32, tag="al")
            nc.scalar.dma_start(al[:], al_d[:])
            vb = cpool.tile([T, T * F], f32r, tag="vb")
            nc.scalar.dma_start(vb[:], vb_d[:])
            at_sb = []
            for kc in range(NT):
                a = cpool.tile([128, N], f32r, tag=f"at{kc}")
                nc.scalar.dma_start(a[:], at_d[kc * 128:(kc + 1) * 128, :])
                at_sb.append(a)

            for b in range(BPC):
                st = spool.tile([T, N], f32r, tag="st")
                nc.sync.dma_start(st[:], stv_d[b])
                xts = []
                for kc in range(NT):
                    xt = xpool.tile([128, T, F], f32r, tag="xt")
                    nc.sync.dma_start(xt[:], x_d[b, kc * 128:(kc + 1) * 128])
                    xts.append(xt)
                for ic in range(NT):
                    pt = ppool.tile([128, T, 1], f32, tag="pt")
                    nc.sync.dma_start(
                        pt[:],
                        sp_d[b, ic * 128:(ic + 1) * 128].rearrange(
                            "p (a b) -> p a b", a=T),
                    )
                    ot = opool.tile([128, T, FA], f32, tag="ot")
                    for nch in range(NCH):
                        ps = pspool.tile([128, 512], f32, tag="ps")
                        for kc in range(NT):
                            nc.tensor.matmul(
                                ps[:],
                                at_sb[kc][:, ic * 128:(ic + 1) * 128],
                                xts[kc].rearrange("p a b -> p (a b)")[
                                    :, nch * 512:(nch + 1) * 512],
                                start=(kc == 0),
                                stop=False,
                            )
                        nc.tensor.matmul(
                            ps[:],
                            st[:, ic * 128:(ic + 1) * 128],
                            vb[:, nch * 512:(nch + 1) * 512],
                            start=False,
                            stop=True,
                        )
                        t0 = nch * TPC
                        # out = alpha*x + psum  (alpha per-partition scalar)
                        nc.vector.scalar_tensor_tensor(
                            ot[:, t0:t0 + TPC, 0:F],
                            xts[ic][:, t0:t0 + TPC, :],
                            al[:, 0:1],
                            ps[:].rearrange("p (a b) -> p a b", a=TPC),
                            mybir.AluOpType.mult,
                            mybir.AluOpType.add,
                        )
                    nc.vector.scalar_tensor_tensor(
                        ot[:, :, F:FA],
                        pt[:].broadcast_to([128, T, NUM_ZEROS]),
                        1.0,
                        rp[:].broadcast_to([128, T, NUM_ZEROS]),
                        mybir.AluOpType.mult,
                        mybir.AluOpType.mult,
                    )
                    nc.scalar.activation(ot[:], ot[:],
                                         mybir.ActivationFunctionType.Relu)
                    nc.scalar.dma_start(out_d[b, ic * 128:(ic + 1) * 128], ot[:])

    nc.compile()
    return nc


def prepare(x, adj, alpha, w, d, w2, d2):
    """Host prep + kernel selection. Returns (nc, in_maps)."""
    import ml_dtypes

    x = np.ascontiguousarray(x, np.float32)
    a = 1.0 / (1.0 + np.exp(-alpha.astype(np.float32)))
    A = 0.125 * a[:, None] * adj.astype(np.float32)

    dc = np.clip(d.astype(np.float32), 0.0, 1.0)
    W = (w.astype(np.float32) * dc) @ w.astype(np.float32).T
    R = W.sum(axis=1)  # [FA]
    d2c = np.clip(d2.astype(np.float32), 0.0, 1.0)
    W2 = (w2.astype(np.float32) * d2c) @ w2.astype(np.float32).T  # [T,T]

    S = x.sum(axis=3)  # [B,N,T]
    rp = np.ascontiguousarray(
        np.broadcast_to(0.25 * R[F:], (128, NUM_ZEROS)), np.float32)

    diag = np.diagonal(W2)
    c = float(diag.mean())
    off = W2 - np.diag(diag)
    is_scaled_identity = (
        np.abs(off).max() <= 1e-6 * max(np.abs(W2).max(), 1e-30)
        and np.abs(diag - c).max() <= 1e-6 * max(abs(c), 1e-30)
    )

    if is_scaled_identity:
        if "fast" not in _CACHE:
            _CACHE["fast"] = _build_fast()
        nc = _CACHE["fast"]
        at = np.ascontiguousarray(A.T)
        stv = np.ascontiguousarray(S.transpose(0, 2, 1))  # [B,T,N]
        vb = np.zeros((T, T * F), np.float32)
        for t in range(T):
            vb[t, t * F:(t + 1) * F] = 0.25 * R[:F]
        al = np.full((128, 1), 0.5 + 0.25 * c, np.float32)
        in_maps = [
            {"xin": x[cr * BPC:(cr + 1) * BPC],
             "at": at,
             "stv": stv[cr * BPC:(cr + 1) * BPC],
             "sp": np.ascontiguousarray(S[cr * BPC:(cr + 1) * BPC]),
             "vb": vb, "al": al, "rp": rp}
            for cr in range(N_CORES)
        ]
        return nc, in_maps

    if "general" not in _CACHE:
        _CACHE["general"] = _build_general()
    nc = _CACHE["general"]
    at = np.ascontiguousarray(A.T, dtype=ml_dtypes.bfloat16)
    q = np.empty((B, N, T, FQ), np.float32)
    xt = np.matmul(x.transpose(0, 1, 3, 2), 0.25 * W2)  # [B,N,F,T]
    q[..., :F] = xt.transpose(0, 1, 3, 2)
    q[..., :F] += 0.5 * x
    q[..., :F] += 0.25 * S[..., None] * R[:F]
    q[..., F] = S
    xb = x.astype(ml_dtypes.bfloat16)
    in_maps = [
        {"xin": xb[cr * BPC:(cr + 1) * BPC], "q": q[cr * BPC:(cr + 1) * BPC],
         "at": at, "rp": rp}
        for cr in range(N_CORES)
    ]
    return nc, in_maps


def kernel(x, adj, alpha, w, d, w2, d2):
    from concourse.bass_utils import run_bass_kernel_spmd

    nc, in_maps = prepare(x, adj, alpha, w, d, w2, d2)
    res = run_bass_kernel_spmd(nc, in_maps, list(range(N_CORES)))
    out = np.concatenate([res.results[c]["out"] for c in range(N_CORES)], axis=0)
    return out
